# revision 26
# baseline (speedup 1.0000x reference)
"""Multi-head attention forward on 8 TRN2 NeuronCores.

Sharding: 8-way tensor-parallel over heads (2 heads per core), both
batches resident on every core. After attention, a per-token-group
AllToAll over all 8 cores redistributes O^T from head-sharded to
token-sharded, and each core runs the full [1024,1024] projection on
its own 512 tokens -- no reduction collective, tiny exposed tail.

Global token index g = b*2048 + t in [0, 4096). Token group i covers
g in [1024i, 1024i+1024); after AllToAll of group i, core c owns the
128-token piece g = 1024i + 128c + [0,128). The host reassembles.

Compute layout is feature-major (transposed) throughout:
  qkT  = W_{q,k}^T @ x^T           [128, 2, 4096] (PE, bf16 in / f32 psum)
  V    = x_tile^T W_v per k-tile   vaug[128, h, kt, 0:64], col 64 = 1.0
  S^T  = kT^T qT per k-tile pair   [128, 1024] psum (two 512-col halves)
  P^T  = exp(S^T / 64)             (ScalarE; no max-subtraction needed:
                                    scores have sigma ~0.125)
  O_aug^T = V_aug^T @ P^T accum    [65, 512] (row 64 = softmax denom)
  epilogue: reciprocal of the denom ROW [1,512] (DVE), gpsimd
  partition_broadcast, one fused multiply into oallT (bf16)
  AllToAll (group i) -> ofull [128, 8, 128] -> y = O^T^T @ W_proj

The S->exp->O chain is software-pipelined: the next pair's S matmuls
are emitted before the previous pair's O matmuls so the in-order PE
queue never waits head-of-line on ScalarE's exp. Each group's proj is
emitted one group late (PE filler) so its AllToAll has time to land.
"""
import os
import sys
import types

import numpy as np

if "/opt/trn_rl_repo" not in sys.path:
    sys.path.insert(0, "/opt/trn_rl_repo")

import concourse.bass as bass
import concourse.bacc as bacc
import concourse.tile as tile
import concourse.mybir as mybir
from concourse.bass_utils import run_bass_kernel_spmd

B, T, D = 2, 2048, 1024
H, HD = 16, 64
N_CORES = 8
GROUP = [list(range(N_CORES))]
HPC = 2                 # heads per core
DSH = HPC * HD          # 128 per-core head features
QKV_COLS = 3 * DSH      # 384
TT = B * T              # 4096 global tokens
TQC = 512               # q-chunk
N_TCH = TT // TQC       # 8 token chunks (qkv phase)
N_KT = T // 128         # 16 k-tiles per batch
N_GRP = 4               # a2a token groups of 1024
GTOK = TT // N_GRP      # 1024

f32 = mybir.dt.float32
bf16 = mybir.dt.bfloat16

LAST_EXEC_NS = None
_CACHE = {}


def _build():
    nc = bacc.Bacc("TRN2", target_bir_lowering=False, debug=False,
                   num_devices=N_CORES)
    xT_ext = nc.dram_tensor("xT", [D, TT], bf16, kind="ExternalInput")
    wqkv_ext = nc.dram_tensor("w_qkv", [D, QKV_COLS], bf16,
                              kind="ExternalInput")
    wproj_ext = nc.dram_tensor("w_proj", [D, D], bf16, kind="ExternalInput")
    out_ext = nc.dram_tensor("out", [TT // N_CORES, D], bf16,
                             kind="ExternalOutput")
    Exp = mybir.ActivationFunctionType.Exp

    with tile.TileContext(nc) as tc:
        with (
            tc.tile_pool(name="persist", bufs=1) as persist,
            tc.tile_pool(name="dram", bufs=1, space="DRAM") as drampool,
        ):
            qkT = persist.tile([128, 2, TT], bf16)    # [2 heads x 64, q|k, g]
            wproj = persist.tile([128, 8, D], bf16)   # fb-blocks of W_proj
            oallT = persist.tile([128, TT], bf16)     # normalized O^T
            vaug = persist.tile([128, HPC, 2 * N_KT, 80], bf16)

            # ship groups: (global token start, width). The last 1024 tokens
            # ship as two 512-token AllToAlls so the exposed tail collective
            # is half-size.
            ships = [(0, 1024), (1024, 1024), (2048, 1024),
                     (3072, 512), (3584, 512)]
            a2a_in = [drampool.tile([D, w // 8], bf16, tag=f"a2ain{i}",
                                    name=f"a2ain{i}")
                      for i, (_, w) in enumerate(ships)]
            a2a_out = [drampool.tile([D, w // 8], bf16, tag=f"a2aout{i}",
                                     name=f"a2aout{i}")
                       for i, (_, w) in enumerate(ships)]

            # ---- phase A: qkv projection (x SBUF-resident, then freed) ----
            with (
                tc.tile_pool(name="qkv_in", bufs=1) as qin,
                tc.tile_pool(name="ps_a", bufs=2, space="PSUM") as psA,
                tc.tile_pool(name="cast_a", bufs=3) as capool,
            ):
                xT = qin.tile([128, 8, TT], bf16)
                wqkv = qin.tile([128, 8, QKV_COLS], bf16)
                wq_src = wqkv_ext.ap().rearrange("(k p) m -> p k m", p=128)
                xT_src = xT_ext.ap().rearrange("(k p) t -> p k t", p=128)
                # order: wq, x0, wk, wv, x1..x7, wproj
                nc.sync.dma_start(wqkv[:, :, 0:128], wq_src[:, :, 0:128])
                nc.sync.dma_start(xT[:, :, 0:TQC], xT_src[:, :, 0:TQC])
                nc.sync.dma_start(wqkv[:, :, 128:256], wq_src[:, :, 128:256])
                nc.sync.dma_start(wqkv[:, :, 256:384], wq_src[:, :, 256:384])
                for tch in range(1, N_TCH):
                    t0 = tch * TQC
                    nc.sync.dma_start(xT[:, :, t0:t0 + TQC],
                                      xT_src[:, :, t0:t0 + TQC])
                nc.sync.dma_start(
                    wproj[:], wproj_ext.ap().rearrange("(c p) d -> p c d",
                                                       p=128))
                nc.gpsimd.memset(vaug[:], 1.0)

                # tiny AllReduce as a cross-core barrier: absorbs the
                # multi-10us core-start skew here, during the input DMA
                # phase, so the first real AllToAll isn't serving as a
                # barrier in the middle of the attention pipeline.
                bar = persist.tile([8, 16], f32, name="bar")
                nc.gpsimd.memset(bar[:], 0.0)
                bar_in = drampool.tile([8, 16], f32, name="bar_in")
                bar_out = drampool.tile([8, 16], f32, name="bar_out")
                nc.sync.dma_start(bar_in[:], bar[:])
                nc.gpsimd.collective_compute(
                    "AllReduce", mybir.AluOpType.add,
                    replica_groups=GROUP, ins=[bar_in[:]], outs=[bar_out[:]])

                for tch in range(N_TCH):
                    t0 = tch * TQC
                    for m in range(2):   # q rows, k rows
                        ps = psA.tile([128, TQC], f32, tag="qkv")
                        for kb in range(8):
                            nc.tensor.matmul(
                                ps[:],
                                wqkv[:, kb, m * 128:(m + 1) * 128],
                                xT[:, kb, t0:t0 + TQC],
                                start=(kb == 0), stop=(kb == 7),
                            )
                        nc.vector.tensor_copy(qkT[:, m, t0:t0 + TQC], ps[:])
                    for kt4 in range(4):
                        kt = tch * 4 + kt4   # global k-tile 0..31
                        vps = psA.tile([128, DSH], f32, tag="v")
                        for kb in range(8):
                            nc.tensor.matmul(
                                vps[:],
                                xT[:, kb, kt * 128:(kt + 1) * 128],
                                wqkv[:, kb, 2 * DSH:3 * DSH],
                                start=(kb == 0), stop=(kb == 7),
                            )
                        nc.vector.tensor_copy(
                            vaug[:, :, kt, 0:HD],
                            vps[:].rearrange("p (h d) -> p h d", d=HD))

            # ---- phase B: attention + AllToAll + proj pipeline ----
            with (
                tc.tile_pool(name="ps_s", bufs=3, space="PSUM") as pss,
                tc.tile_pool(name="ps_o", bufs=2, space="PSUM") as pso,
                tc.tile_pool(name="attn", bufs=4) as apool,
                tc.tile_pool(name="attn2", bufs=2) as apool2,
                tc.tile_pool(name="proj", bufs=2) as ppool,
            ):
                def epilogue(g0, h, o_ps):
                    """Normalize head h's O into oallT[64h:64h+64, g0:g0+512].
                    Fast approx reciprocal on the single denom row (~51 ULP,
                    safe: denoms ~2048), broadcast on GpSimd, one fused
                    multiply on DVE."""
                    rrow = apool2.tile([1, TQC], f32, tag="rrow")
                    nc.vector.tensor_copy(rrow[:], o_ps[HD:HD + 1, :])
                    rinv = apool2.tile([1, TQC], f32, tag="rinv")
                    nc.vector.reciprocal_approx_fast(rinv[:], rrow[:])
                    rb = apool2.tile([HD, TQC], f32, tag="rb")
                    nc.gpsimd.partition_broadcast(rb[:], rinv[:])
                    nc.vector.tensor_tensor(
                        out=oallT[h * HD:(h + 1) * HD, g0:g0 + TQC],
                        in0=o_ps[0:HD, :], in1=rb[:],
                        op=mybir.AluOpType.mult)

                RUNAHEAD = 2   # k-tile PAIRS of S/exp in flight ahead of O

                def attn_unit(un, filler=None):
                    """Unit un in 0..15: 512-token q-chunk un//2, head un%2.
                    Pair-level software pipeline: S(j) and exp(j) run
                    RUNAHEAD pairs ahead of O(j) (~2us of queued PE work vs
                    ~1.3us exp latency) so the in-order PE queue never
                    stalls on ScalarE -- micro-stalls drop the PE out of its
                    max p-state and halve its clock. `filler` (extra PE
                    work, e.g. a proj half) is emitted mid-unit."""
                    g0 = (un // 2) * TQC
                    b = g0 // T
                    h = un % 2
                    hp = h * HD
                    kbase = b * T // 128   # first global k-tile of batch b
                    o_ps = pso.tile([HD + 1, TQC], f32, tag="o")
                    NP = N_KT // 2
                    pk = [None] * NP

                    def s_exp(j):
                        s2 = pss.tile([128, 2 * TQC], f32, tag="s")
                        for half in range(2):
                            kg = (kbase + 2 * j + half) * 128
                            nc.tensor.matmul(
                                s2[:, half * TQC:(half + 1) * TQC],
                                qkT[hp:hp + HD, 1, kg:kg + 128],
                                qkT[hp:hp + HD, 0, g0:g0 + TQC],
                                start=True, stop=True,
                            )
                        p2 = apool.tile([128, 2 * TQC], bf16, tag="p")
                        nc.scalar.activation(p2[:], s2[:], Exp,
                                             scale=1.0 / HD)
                        pk[j] = p2

                    def o_mm(j):
                        for half in range(2):
                            kt = 2 * j + half
                            nc.tensor.matmul(
                                o_ps[:], vaug[:, h, kbase + kt, 0:HD + 1],
                                pk[j][:, half * TQC:(half + 1) * TQC],
                                start=(kt == 0), stop=(kt == N_KT - 1),
                            )

                    for j in range(NP):
                        s_exp(j)
                        if j == NP // 2 and filler is not None:
                            filler()
                        if j >= RUNAHEAD:
                            o_mm(j - RUNAHEAD)
                    for j in range(NP - RUNAHEAD, NP):
                        o_mm(j)
                    return g0, h, o_ps

                # out_ext row offset of each ship's owned piece
                ship_rows = [0]
                for _, w in ships:
                    ship_rows.append(ship_rows[-1] + w // 8)

                def ship(si):
                    """oallT slice -> dram (split by dest core) -> AllToAll"""
                    gb, w = ships[si]
                    pw = w // 8   # per-core token piece
                    nc.sync.dma_start(
                        a2a_in[si].rearrange("(c f) t -> f c t", f=128),
                        oallT[:, gb:gb + w].rearrange(
                            "f (c t) -> f c t", t=pw))
                    nc.gpsimd.collective_compute(
                        "AllToAll", mybir.AluOpType.bypass,
                        replica_groups=GROUP,
                        ins=[a2a_in[si][:]],
                        outs=[a2a_out[si][:]],
                    )

                def proj(si, half):
                    """Half (512 out-cols) of ship si's projection."""
                    pw = ships[si][1] // 8
                    if half == 0:
                        ofull = ppool.tile([128, 8, 128], bf16,
                                           tag="ofull", name="ofull")
                        nc.sync.dma_start(
                            ofull[:, :, 0:pw],
                            a2a_out[si].rearrange("(c f) t -> f c t", f=128))
                        y_sb = ppool.tile([128, D], bf16, tag="ysb",
                                          name="ysb")
                        proj.cur = (ofull, y_sb)
                    ofull, y_sb = proj.cur
                    nn = half
                    # borrow a ps_s ring buffer (PSUM banks are all spoken
                    # for); use its first 512 columns
                    y_ps = pss.tile([128, 2 * TQC], f32,
                                    tag="s", name="y_ps")[:, 0:512]
                    for fb in range(8):
                        nc.tensor.matmul(
                            y_ps[0:pw, :],
                            ofull[:, fb, 0:pw],
                            wproj[:, fb, nn * 512:(nn + 1) * 512],
                            start=(fb == 0), stop=(fb == 7),
                        )
                    nc.vector.tensor_copy(
                        y_sb[0:pw, nn * 512:(nn + 1) * 512], y_ps[0:pw, :])
                    if half == 1:
                        r0 = ship_rows[si]
                        nc.sync.dma_start(
                            out_ext.ap()[r0:r0 + pw, :], y_sb[0:pw, :])

                # unit index after which each ship's tokens are complete
                ship_after = {3: 0, 7: 1, 11: 2, 13: 3, 15: 4}
                # (unit -> ship, half) proj filler slots: ship si's proj
                # runs ~3 units after its AllToAll was issued so even a
                # slow collective never head-of-line blocks the PE queue
                proj_slots = {6: (0, 0), 7: (0, 1), 10: (1, 0), 11: (1, 1),
                              14: (2, 0), 15: (2, 1)}

                for un in range(16):
                    slot = proj_slots.get(un)
                    filler = (lambda s=slot: proj(*s)) if slot else None
                    res = attn_unit(un, filler)
                    epilogue(*res)
                    if un in ship_after:
                        ship(ship_after[un])
                proj(3, 0)
                proj(3, 1)
                proj(4, 0)
                proj(4, 1)

    nc.compile()
    return nc


def _install_profile_hook():
    """Provide antenv.axon_hooks (absent in this image) so bass_utils'
    axon trace path can reach the NTFF profiler in libaxon_pjrt.so."""
    try:
        import antenv
        if "antenv.axon_hooks" not in sys.modules:
            mod = types.ModuleType("antenv.axon_hooks")
            mod._hook = None
            mod.set_axon_ntff_profile_hook = lambda h: setattr(mod, "_hook", h)
            mod.get_axon_ntff_profile_hook = lambda: mod._hook
            sys.modules["antenv.axon_hooks"] = mod
            antenv.axon_hooks = mod
        from trn_agent_boot.trn_boot import _ntff_profile_via_ctypes
        hook = _ntff_profile_via_ctypes("/opt/axon/libaxon_pjrt.so")
        sys.modules["antenv.axon_hooks"].set_axon_ntff_profile_hook(hook)
        return True
    except Exception:
        return False


def kernel(x, W_qkv, W_proj):
    global LAST_EXEC_NS
    x = np.asarray(x, dtype=np.float32)
    W_qkv = np.asarray(W_qkv, dtype=np.float32)
    W_proj = np.asarray(W_proj, dtype=np.float32)

    if "nc" not in _CACHE:
        _CACHE["nc"] = _build()
    nc = _CACHE["nc"]

    npbf16 = mybir.dt.np(bf16)
    xT = np.ascontiguousarray(x.reshape(TT, D).T).astype(npbf16)
    wproj = W_proj.astype(npbf16)
    in_maps = []
    for c in range(N_CORES):
        f0 = c * DSH
        wq = W_qkv[:, f0:f0 + DSH]
        wk = W_qkv[:, D + f0:D + f0 + DSH]
        wv = W_qkv[:, 2 * D + f0:2 * D + f0 + DSH]
        in_maps.append({
            "xT": xT,
            "w_qkv": np.concatenate([wq, wk, wv], axis=1).astype(npbf16),
            "w_proj": wproj,
        })

    profile = bool(os.environ.get("BASS_KERNEL_PROFILE"))
    trace_dir = os.environ.get("BASS_KERNEL_TRACE_DIR") or None
    if profile:
        profile = _install_profile_hook()
    res = run_bass_kernel_spmd(
        nc, in_maps, core_ids=list(range(N_CORES)),
        trace=profile, tmpdir=trace_dir)
    LAST_EXEC_NS = res.exec_time_ns

    ships = [(0, 1024), (1024, 1024), (2048, 1024), (3072, 512), (3584, 512)]
    y = np.empty((B, T, D), dtype=np.float32)
    for c in range(N_CORES):
        oc = res.results[c]["out"].astype(np.float32)
        r0 = 0
        for gb, w in ships:
            pw = w // 8
            g0 = gb + c * pw
            b, t0 = g0 // T, g0 % T
            y[b, t0:t0 + pw, :] = oc[r0:r0 + pw, :]
            r0 += pw
    return y
